# revision 29
# baseline (speedup 1.0000x reference)
"""Trainium2 8-core kernel for the AGI transformer block.

Sharding: 2-way data parallel over batch x 4-way tensor parallel over heads.
Core c: batch b=c//4, feature band g=c%4 (256 features = 4 main heads of 64 /
1 causal head of 256 / 1 meta head of 256).

Per core (band slice G = [256g, 256g+256)):
  - main attention: 4 heads, q pre-scaled 1/8, sigmoid(gate+aw) modulation
    folded into q per-head; rowsums via ones-column in augmented V (M=65);
    head pairs share the PE array via base-partition 0/64 row packing.
  - causal MHA head: hd=256, q pre-scaled 1/16; 0.9 blend folded into out-proj
    weight; main's 0.1-scaled ctx placed into the out-proj PSUM via a host-built
    placement matrix (keeps the SPMD program core-independent).
  - blend combine: ReduceScatter(add) -> own band [256,2048] (for the final
    0.85-term) + AllGather -> full blended ctx [1024,2048] (for meta).
  - meta MHA head: hd=256; 0.15*meta_out_w.T@out_w.T folded into one matrix.
  - final: outP = mowT.T@metaA + owT.T@band_ctx  (partial; host sums 4 cores).

Storage is bf16 (halves SBUF + HBM traffic); accumulation is f32 in PSUM.
Emission interleaves ACT-bound main attention with PE-bound causal attention
and meta projections so the TensorE stream stays dense (HAM stays warm).
Chunked tiles keep Tile's dependency tracking precise so collectives overlap.
"""

import ml_dtypes
import numpy as np

import concourse.mybir as mybir
import concourse.tile as tile
from concourse import bacc
from concourse.bass_utils import run_bass_kernel_spmd

F32 = mybir.dt.float32
BF16 = mybir.dt.bfloat16
AF = mybir.ActivationFunctionType
BF = ml_dtypes.bfloat16

B, S, D = 2, 2048, 1024
NCORES = 8
G = 4  # tensor-parallel group size
BAND = 256  # features per core
IC, NIC = 512, 4  # i-chunk (query) tiling
NJT = 16  # j tiles of 128
NKT = 8  # contraction tiles of 128 over D
CAUSAL_ACTIVE = 0.9
MW = ((0.9 - 0.8) / 0.2) * 0.3  # 0.15


def build_program():
    nc = bacc.Bacc("TRN2", target_bir_lowering=False, debug=False,
                   num_devices=NCORES)

    def din(name, shape):
        return nc.dram_tensor(name, shape, BF16, kind="ExternalInput").ap()

    xT = din("xT", [D, S])
    wqT = din("wqT", [D, BAND])
    wkT = din("wkT", [D, BAND])
    wvT = din("wvT", [D, 260])  # 4x(64 head cols + zero col for ones)
    gwT = din("gwT", [D, 4])
    selT = din("selT", [4, 512])  # 4 one-hot row-selector blocks [4,128]
    awc = nc.dram_tensor("awc", [1, 4], F32, kind="ExternalInput").ap()
    cqT = din("cqT", [D, BAND])
    ckT = din("ckT", [D, BAND])
    cvT = din("cvT", [D, BAND])
    cowT = din("cowT", [BAND, D])
    pcT = din("pcT", [BAND, D])  # placement matrix (0.1 at own band)
    pselT = din("pselT", [D, BAND])  # one-hot band row-selector (chunk 3)
    mqT = din("mqT", [D, BAND])
    mkT = din("mkT", [D, BAND])
    mvT = din("mvT", [D, BAND])
    mowT = din("mowT", [BAND, D])
    owT = din("owT", [BAND, D])
    outP = nc.dram_tensor("outP", [D, S], F32, kind="ExternalOutput").ap()

    groups = [[0, 1, 2, 3], [4, 5, 6, 7]]

    with tile.TileContext(nc) as tc:
        with (
            tc.tile_pool(name="wts", bufs=1) as wts,
            tc.tile_pool(name="act", bufs=1) as actp,
            tc.tile_pool(name="small", bufs=1) as small,
            tc.tile_pool(name="work", bufs=3) as work,
            tc.tile_pool(name="stat", bufs=2) as statp,
            tc.tile_pool(name="psE", bufs=3, space="PSUM") as psE,
            tc.tile_pool(name="psA", bufs=4, space="PSUM") as psA,
            tc.tile_pool(name="psR", bufs=1, space="PSUM") as psR,
            tc.tile_pool(name="dram", bufs=1, space="DRAM") as dram,
        ):
            def load_w(name, ap, cols, tag):
                t = wts.tile([128, NKT, cols], BF16, name=name, tag=tag)
                for kt in range(NKT):
                    nc.sync.dma_start(t[:, kt, :],
                                      ap[kt * 128:(kt + 1) * 128, :])
                return t

            def load_w2(name, ap, tag):  # [256, 1024] -> [128, 2, 1024]
                t = wts.tile([128, 2, D], BF16, name=name, tag=tag)
                for kt in range(2):
                    nc.sync.dma_start(t[:, kt, :],
                                      ap[kt * 128:(kt + 1) * 128, :])
                return t

            wq_sb = load_w("wq_sb", wqT, BAND, "wq")

            # xT per-kt tiles; tags pair them with later-stage tiles so the
            # SBUF slots time-share (xT dies before those are written)
            xtags = ["ctxC0", "ctxC1", "ctxC2", "ctxC3",
                     "qT2", "kT2", "vA2", "bandC0"]
            xT_t = []
            for kt in range(NKT):
                t = actp.tile([128, S], BF16, name=f"xTt{kt}", tag=xtags[kt])
                nc.sync.dma_start(t[:, :], xT[kt * 128:(kt + 1) * 128, :])
                xT_t.append(t)

            wk_sb = load_w("wk_sb", wkT, BAND, "wk")
            wv_sb = load_w("wv_sb", wvT, 260, "wv")
            gw_sb = load_w("gw_sb", gwT, 4, "gw")
            cq_sb = load_w("cq_sb", cqT, BAND, "cq")
            ck_sb = load_w("ck_sb", ckT, BAND, "ck")
            cv_sb = load_w("cv_sb", cvT, BAND, "cv")
            cow_sb = load_w2("cow_sb", cowT, "cow")
            pc_sb = load_w2("pc_sb", pcT, "pc")
            mow_sb = load_w2("mow_sb", mowT, "mow")  # own slots: load early
            ow_sb = load_w2("ow_sb", owT, "ow")

            aw_sb = small.tile([4, 1], F32)
            nc.sync.dma_start(aw_sb[:, :], awc.rearrange("a b -> b a"))
            sel_sb = small.tile([4, 512], BF16)
            nc.sync.dma_start(sel_sb[:, :], selT[:, :])
            ones_sb = small.tile([128, 1], BF16)
            nc.vector.memset(ones_sb[:, :], 1.0)
            onesrow = small.tile([1, 128], BF16)
            nc.vector.memset(onesrow[:, :], 1.0)

            # ---------- projections ----------
            qT_sb = actp.tile([128, 2, S], BF16, tag="qT")
            kT_sb = actp.tile([128, 2, S], BF16, tag="kT")

            def proj_chunk(dst, w_sb, src_t, ot, icc):
                ps = psA.tile([128, IC], F32, tag="acc")
                for kt in range(NKT):
                    nc.tensor.matmul(
                        ps[:, :],
                        w_sb[:, kt, ot * 128:(ot + 1) * 128],
                        src_t[kt][:, icc * IC:(icc + 1) * IC],
                        start=(kt == 0), stop=(kt == NKT - 1))
                nc.vector.tensor_copy(dst[:, ot, icc * IC:(icc + 1) * IC],
                                      ps[:, :])

            def proj_T(dst, w_sb, src_t):  # dst [128, 2, S]
                for ot in range(2):
                    for icc in range(NIC):
                        proj_chunk(dst, w_sb, src_t, ot, icc)

            # gate matmuls + sigmoid for ALL chunks now (keeps the sigmoid
            # table-set switch out of the attention blocks)
            mrow4 = small.tile([4, S], BF16)
            for icc in range(NIC):
                i0 = icc * IC
                g_ps = psR.tile([4, IC], F32, tag="rs")
                for kt in range(NKT):
                    nc.tensor.matmul(g_ps[:, :],
                                     gw_sb[:, kt, 0:4],
                                     xT_t[kt][:, i0:i0 + IC],
                                     start=(kt == 0), stop=(kt == NKT - 1))
                nc.scalar.activation(mrow4[:, i0:i0 + IC], g_ps[:, :],
                                     AF.Sigmoid, bias=aw_sb[:, 0:1], scale=1.0)

            qs_sb = actp.tile([128, 2, 2 * IC], BF16, tag="qs")  # 2-chunk ring

            def qmod(h, icc):
                # broadcast row h of mrow4 to 128 partitions via a K=4 matmul
                # against a host-provided one-hot selector, then scale q into
                # a separate tile (avoids write-after-read on qT)
                rh, oh = (h % 2) * 64, h // 2
                i0 = icc * IC
                pb = psR.tile([128, IC], F32, tag="rs")
                nc.tensor.matmul(pb[:, :],
                                 sel_sb[0:4, h * 128:(h + 1) * 128],
                                 mrow4[0:4, i0:i0 + IC])
                r0 = (icc % 2) * IC
                nc.vector.tensor_mul(qs_sb[rh:rh + 64, oh, r0:r0 + IC],
                                     qT_sb[rh:rh + 64, oh, i0:i0 + IC],
                                     pb[rh:rh + 64, :])

            def qproj_steps(icc):
                """q + cq projection/modulation filler steps for chunk icc"""
                steps = []
                for ot in range(2):
                    def sq(ot=ot, icc=icc):
                        proj_chunk(qT_sb, wq_sb, xT_t, ot, icc)
                        qmod(2 * ot, icc)
                        qmod(2 * ot + 1, icc)
                    steps.append(sq)
                for ot in range(2):
                    steps.append(lambda ot=ot, icc=icc: proj_chunk(
                        cqT_sb, cq_sb, xT_t, ot, icc))
                return steps

            # stage B: only chunk 0 of q/cq; full k/v/ck/cv
            cqT_sb = actp.tile([128, 2, S], BF16, tag="cqT")
            for st_ in qproj_steps(0):
                st_()
            proj_T(kT_sb, wk_sb, xT_t)

            # v natural layout [2048 j, 260]: ones cols at 64,129,194,259
            v_sb = actp.tile([128, NJT, 260], BF16, tag="vA")
            for st in range(NJT):
                ps = psA.tile([128, 260], F32, tag="acc")
                for kt in range(NKT):
                    nc.tensor.matmul(ps[:, :],
                                     xT_t[kt][:, st * 128:(st + 1) * 128],
                                     wv_sb[:, kt, :],
                                     start=(kt == 0), stop=(kt == NKT - 1))
                nc.vector.tensor_copy(v_sb[:, st, :], ps[:, :])
                nc.vector.memset(v_sb[:, st, 64:260:65], 1.0)

            ckT_sb = actp.tile([128, 2, S], BF16, tag="ckT")
            proj_T(ckT_sb, ck_sb, xT_t)

            cv_nat = actp.tile([128, NJT, BAND], BF16, tag="cvN")
            for st in range(NJT):
                ps = psA.tile([128, BAND], F32, tag="acc")
                for kt in range(NKT):
                    nc.tensor.matmul(ps[:, :],
                                     xT_t[kt][:, st * 128:(st + 1) * 128],
                                     cv_sb[:, kt, :],
                                     start=(kt == 0), stop=(kt == NKT - 1))
                nc.vector.tensor_copy(cv_nat[:, st, :], ps[:, :])

            # meta weights: load now (slots of wq/wk/wv just freed; sync queue
            # still shallow, so they land long before the meta stage)
            mq_sb = load_w("mq_sb", mqT, BAND, "wq")
            mk_sb = load_w("mk_sb", mkT, BAND, "wk")
            mv_sb = load_w("mv_sb", mvT, BAND, "wv")

            # ---------- chunked tiles ----------
            ctxm_sb = actp.tile([128, 2, S], BF16, tag="ctxm")  # 0.1*main ctx
            cA_sb = actp.tile([128, 2, S], BF16, tag="cA")
            ctxC = [actp.tile([128, NKT, IC], BF16, name=f"ctxC{i}",
                              tag=f"ctxC{i}") for i in range(NIC)]
            bandC = [actp.tile([128, 2, IC], BF16, name=f"bandC{i}",
                               tag=f"bandC{i}") for i in range(NIC)]
            mqT_sb = actp.tile([128, 2, S], BF16, tag="qT2")
            mkT_sb = actp.tile([128, 2, S], BF16, tag="kT2")
            mv_nat = actp.tile([128, NJT, BAND], BF16, tag="vA2")

            arB, rsO, agO, arO3 = [], [], [], []
            for icc in range(NIC):
                nh = 1 if icc < NIC - 1 else 2
                arB.append([dram.tile([D, IC // nh], BF16,
                                      name=f"arB{icc}_{hh}", tag=f"arB{icc}{hh}")
                            for hh in range(nh)])
                rsO.append([dram.tile([BAND, IC // nh], BF16,
                                      name=f"rsO{icc}_{hh}", tag=f"rsO{icc}{hh}")
                            for hh in range(nh)])
                if icc == NIC - 1:
                    arO3.extend([dram.tile([D, IC // nh], BF16,
                                           name=f"arO3_{hh}", tag=f"arO3{hh}")
                                 for hh in range(nh)])
                agO.append([dram.tile([D, IC // nh], BF16,
                                      name=f"agO{icc}_{hh}", tag=f"agO{icc}{hh}")
                            for hh in range(nh)])

            def div_batch(specs, i0):
                """batched softmax divisions: pipeline ln -> exp -> bcast ->
                muls across several heads so chain latencies overlap.
                spec: ("head", h, acc) or ("wide", dst_sb, a1, a2, rs)"""
                lnrs = []
                for sp in specs:
                    lnr = statp.tile([1, IC], F32, tag="lnr", bufs=3)
                    src = sp[2][64:65, :] if sp[0] == "head" else sp[4][:, :]
                    nc.scalar.activation(lnr[:, :], src, AF.Ln)
                    lnrs.append(lnr)
                rcps = []
                for sp, lnr in zip(specs, lnrs):
                    rcp = statp.tile([1, IC], BF16, tag="rcp", bufs=3)
                    nc.scalar.activation(rcp[:, :], lnr[:, :], AF.Exp,
                                         scale=-1.0)
                    rcps.append(rcp)
                pbs = []
                for sp, rcp in zip(specs, rcps):
                    n = 64 if sp[0] == "head" else 128
                    pb_ps = psE.tile([128, IC], F32, tag="eps")
                    nc.tensor.matmul(pb_ps[:, :], onesrow[0:1, :], rcp[:, :])
                    pb = work.tile([n, IC], BF16,
                                   tag="pbm" if n == 64 else "pb2", bufs=3)
                    nc.scalar.copy(pb[:, :], pb_ps[0:n, :])
                    pbs.append(pb)
                for sp, pb in zip(specs, pbs):
                    if sp[0] == "head":
                        h, acc = sp[1], sp[2]
                        rh, oh = (h % 2) * 64, h // 2
                        nc.vector.tensor_mul(
                            ctxm_sb[rh:rh + 64, oh, i0:i0 + IC],
                            acc[0:64, :], pb[:, :])
                    else:
                        dst_sb, a1, a2 = sp[1], sp[2], sp[3]
                        nc.vector.tensor_mul(dst_sb[:, 0, i0:i0 + IC],
                                             a1[:, :], pb[:, :])
                        nc.vector.tensor_mul(dst_sb[:, 1, i0:i0 + IC],
                                             a2[:, :], pb[:, :])

            def main_pair_step(p, jt, i0, accs):
                """one j-tile for main head pair p (heads 2p, 2p+1): the two
                E matmuls row-pack (base partitions 0/64) and run concurrently"""
                oh = p
                esbs = []
                for hh in range(2):
                    rh = hh * 64
                    eps = psE.tile([128, IC], F32, tag="eps")
                    r0 = (i0 // IC % 2) * IC
                    nc.tensor.matmul(
                        eps[:, :],
                        kT_sb[rh:rh + 64, oh, jt * 128:(jt + 1) * 128],
                        qs_sb[rh:rh + 64, oh, r0:r0 + IC])
                    esb = work.tile([128, IC], BF16, tag="esb", bufs=4)
                    nc.scalar.activation(esb[:, :], eps[:, :], AF.Exp)
                    esbs.append(esb)
                for hh in range(2):
                    h = 2 * p + hh
                    nc.tensor.matmul(
                        accs[hh][:, :],
                        v_sb[:, jt, h * 65:h * 65 + 65],
                        esbs[hh][:, :],
                        start=(jt == 0), stop=(jt == NJT - 1))

            def wide_attn_step(kTt, qTt, vnat, jt, i0, a1, a2, rs):
                """one j-tile of a hd-256 attention (causal or meta)"""
                eps = psE.tile([128, IC], F32, tag="eps")
                for dkt in range(2):
                    nc.tensor.matmul(
                        eps[:, :],
                        kTt[:, dkt, jt * 128:(jt + 1) * 128],
                        qTt[:, dkt, i0:i0 + IC],
                        start=(dkt == 0), stop=(dkt == 1))
                esb = work.tile([128, IC], BF16, tag="esb", bufs=4)
                nc.scalar.activation(esb[:, :], eps[:, :], AF.Exp)
                st_, sp_ = (jt == 0), (jt == NJT - 1)
                nc.tensor.matmul(a1[:, :], vnat[:, jt, 0:128], esb[:, :],
                                 start=st_, stop=sp_)
                nc.tensor.matmul(a2[:, :], vnat[:, jt, 128:256], esb[:, :],
                                 start=st_, stop=sp_)
                nc.tensor.matmul(rs[:, :], ones_sb[:, 0:1], esb[:, :],
                                 start=st_, stop=sp_)

            def metaproj_steps(icc):
                """closures, each emitting one PSUM group of chunk icc's meta
                projections (interleaved into phase 2 of chunk icc+1)"""
                i0 = icc * IC
                steps = []

                def projstep(w_sb, dst, ot, i0=i0, icc=icc):
                    ps = psA.tile([128, IC], F32, tag="acc")
                    for kt in range(NKT):
                        nc.tensor.matmul(
                            ps[:, :], w_sb[:, kt, ot * 128:(ot + 1) * 128],
                            ctxC[icc][:, kt, :],
                            start=(kt == 0), stop=(kt == NKT - 1))
                    nc.vector.tensor_copy(dst[:, ot, i0:i0 + IC], ps[:, :])

                def vstep(st4, icc=icc):
                    st = icc * 4 + st4
                    ps = psA.tile([128, BAND], F32, tag="acc")
                    for kt in range(NKT):
                        nc.tensor.matmul(
                            ps[:, :],
                            ctxC[icc][:, kt, st4 * 128:(st4 + 1) * 128],
                            mv_sb[:, kt, :],
                            start=(kt == 0), stop=(kt == NKT - 1))
                    nc.vector.tensor_copy(mv_nat[:, st, :], ps[:, :])

                for ot in range(2):
                    steps.append(lambda ot=ot: projstep(mq_sb, mqT_sb, ot))
                for ot in range(2):
                    steps.append(lambda ot=ot: projstep(mk_sb, mkT_sb, ot))
                for st4 in range(4):
                    steps.append(lambda st4=st4: vstep(st4))
                return steps

            # ---------- per-chunk pipeline ----------
            pending_rdback = []
            for icc in range(NIC):
                i0 = icc * IC
                while pending_rdback:
                    pending_rdback.pop(0)()
                # phase 1: main heads (0,1) interleaved with causal attention
                accA = [psA.tile([65, IC], F32, tag="acc", name=f"accA{icc}{i}")
                        for i in range(2)]
                ca1 = psA.tile([128, IC], F32, tag="acc")
                ca2 = psA.tile([128, IC], F32, tag="acc")
                crs = psR.tile([1, IC], F32, tag="rs")
                for jt in range(NJT):
                    main_pair_step(0, jt, i0, accA)
                    wide_attn_step(ckT_sb, cqT_sb, cv_nat, jt, i0, ca1, ca2, crs)
                div_batch([("head", 0, accA[0]), ("head", 1, accA[1]),
                           ("wide", cA_sb, ca1, ca2, crs)], i0)

                # phase 2: main heads (2,3) interleaved with filler PE work:
                # next chunk's q/cq projections + older chunk's meta projs
                accB = [psA.tile([65, IC], F32, tag="acc", name=f"accB{icc}{i}")
                        for i in range(2)]
                fillers = []
                if icc + 1 < NIC:
                    fillers += qproj_steps(icc + 1)
                if icc >= 2:
                    fillers += metaproj_steps(icc - 2)
                # weave ~2/3 of the fillers into rounds 0..11 so the pair-B
                # accumulators stop promptly; the rest interleave with the
                # causal out-proj below (keeps PE dense across the boundary)
                nfront = len(fillers)
                done = 0
                for jt in range(NJT):
                    main_pair_step(1, jt, i0, accB)
                    want = (jt + 1) * nfront // NJT
                    while done < want:
                        fillers[done]()
                        done += 1
                div_batch([("head", 2, accB[0]), ("head", 3, accB[1])], i0)

                # causal out-proj + main placement -> arB chunk
                for ot in range(8):
                    ps = psA.tile([128, IC], F32, tag="acc")
                    for ft in range(2):
                        nc.tensor.matmul(
                            ps[:, :],
                            cow_sb[:, ft, ot * 128:(ot + 1) * 128],
                            cA_sb[:, ft, i0:i0 + IC],
                            start=(ft == 0), stop=False)
                    for rt in range(2):
                        nc.tensor.matmul(
                            ps[:, :],
                            pc_sb[:, rt, ot * 128:(ot + 1) * 128],
                            ctxm_sb[:, rt, i0:i0 + IC],
                            start=False, stop=(rt == 1))
                    ob = work.tile([128, IC], BF16, tag="obA", bufs=2)
                    nc.vector.tensor_copy(ob[:, :], ps[:, :])
                    nh = len(arB[icc])
                    hw_ = IC // nh
                    for hh in range(nh):
                        nc.sync.dma_start(
                            arB[icc][hh][ot * 128:(ot + 1) * 128, :],
                            ob[:, hh * hw_:(hh + 1) * hw_])
                    if done < len(fillers) and ot % 2 == 1:
                        fillers[done]()
                        done += 1

                # blend combine: RS (own band) + AG (full ctx); the last
                # chunk is split in half so its latency tail is shorter.
                # Read-back DMAs are DEFERRED one block so the sync queue
                # never parks on an unfinished collective (head-of-line).
                nh = len(arB[icc])
                hw_ = IC // nh
                for hh in range(nh):
                    c0 = hh * hw_
                    if icc < NIC - 1:
                        nc.gpsimd.collective_compute(
                            "ReduceScatter", mybir.AluOpType.add,
                            replica_groups=groups,
                            ins=[arB[icc][hh][:, :].opt()],
                            outs=[rsO[icc][hh][:, :].opt()])
                        nc.gpsimd.collective_compute(
                            "AllGather", mybir.AluOpType.bypass,
                            replica_groups=groups,
                            ins=[rsO[icc][hh][:, :].opt()],
                            outs=[agO[icc][hh][:, :].opt()])
                    else:
                        # last chunk: single AllReduce per half (shorter
                        # serial chain on the collective engine); own band
                        # recovered by a select matmul in the tail
                        nc.gpsimd.collective_compute(
                            "AllReduce", mybir.AluOpType.add,
                            replica_groups=groups,
                            ins=[arB[icc][hh][:, :].opt()],
                            outs=[arO3[hh][:, :].opt()])

                    def rdback(icc=icc, hh=hh, c0=c0, hw_=hw_):
                        src = agO[icc][hh] if icc < NIC - 1 else arO3[hh]
                        for kt in range(NKT):
                            nc.sync.dma_start(
                                ctxC[icc][:, kt, c0:c0 + hw_],
                                src[kt * 128:(kt + 1) * 128, :])
                        if icc < NIC - 1:
                            for kt in range(2):
                                nc.sync.dma_start(
                                    bandC[icc][:, kt, c0:c0 + hw_],
                                    rsO[icc][hh][kt * 128:(kt + 1) * 128, :])
                    pending_rdback.append(rdback)

            # meta projections for the last two chunks (chunk 2 overlaps
            # the RS3/AG3 tail; chunk 3 is emitted inside the first meta
            # attention chunk below, after its early j-tiles)
            while pending_rdback:
                pending_rdback.pop(0)()
            for st in metaproj_steps(NIC - 2):
                st()

            # ---------- meta attention + final out-proj ----------
            def final_steps(icc):
                i0 = icc * IC
                steps = []

                def fstep(ot, icc=icc, i0=i0):
                    ps = psA.tile([128, IC], F32, tag="acc")
                    for ft in range(2):
                        nc.tensor.matmul(
                            ps[:, :],
                            mow_sb[:, ft, ot * 128:(ot + 1) * 128],
                            mA_sb[:, ft, i0:i0 + IC],
                            start=(ft == 0), stop=False)
                    for ft in range(2):
                        nc.tensor.matmul(
                            ps[:, :],
                            ow_sb[:, ft, ot * 128:(ot + 1) * 128],
                            bandC[icc][:, ft, :],
                            start=False, stop=(ft == 1))
                    ob = work.tile([128, IC], F32, tag="obF", bufs=2)
                    nc.vector.tensor_copy(ob[:, :], ps[:, :])
                    nc.sync.dma_start(
                        outP[ot * 128:(ot + 1) * 128, i0:i0 + IC], ob[:, :])
                for ot in range(8):
                    steps.append(lambda ot=ot: fstep(ot))
                return steps

            mA_sb = actp.tile([128, 2, S], BF16, tag="cqT")  # reuse slot
            for icc in range(NIC):
                i0 = icc * IC
                fsteps = final_steps(icc - 1) if icc > 0 else []
                a1 = psA.tile([128, IC], F32, tag="acc")
                a2 = psA.tile([128, IC], F32, tag="acc")
                rs = psR.tile([1, IC], F32, tag="rs")
                # j-tiles 0..11 only touch chunks 0-2 of mk/mv, so the first
                # i-chunk's early j-tiles run while chunk 3's AG completes;
                # chunk 3's meta projections emit before the last 4 j-tiles
                for jt in range(12):
                    wide_attn_step(mkT_sb, mqT_sb, mv_nat, jt, i0, a1, a2, rs)
                    if fsteps and jt % 2 == 1 and jt // 2 < len(fsteps):
                        fsteps[jt // 2]()
                if icc == 0:
                    psel_sb = load_w("psel_sb", pselT, BAND, "cq")
                    for st in metaproj_steps(NIC - 1):
                        st()
                    for rt in range(2):
                        ps = psA.tile([128, IC], F32, tag="acc")
                        for kt in range(NKT):
                            nc.tensor.matmul(
                                ps[:, :],
                                psel_sb[:, kt, rt * 128:(rt + 1) * 128],
                                ctxC[NIC - 1][:, kt, :],
                                start=(kt == 0), stop=(kt == NKT - 1))
                        nc.vector.tensor_copy(bandC[NIC - 1][:, rt, :],
                                              ps[:, :])
                for jt in range(12, NJT):
                    wide_attn_step(mkT_sb, mqT_sb, mv_nat, jt, i0, a1, a2, rs)
                    if fsteps and jt % 2 == 1 and jt // 2 < len(fsteps):
                        fsteps[jt // 2]()
                div_batch([("wide", mA_sb, a1, a2, rs)], i0)

            for st in final_steps(NIC - 1):
                st()

    nc.compile()
    return nc


_NC = None


def _get_nc():
    global _NC
    if _NC is None:
        _NC = build_program()
    return _NC


def kernel(hidden_states, consciousness_vector, wq, bq, wk, bk, wv, bv,
           gate_w, gate_b, aw_w, aw_b,
           causal_in_w, causal_in_b, causal_out_w, causal_out_b,
           meta_in_w, meta_in_b, meta_out_w, meta_out_b,
           out_w, out_b):
    f = np.float32
    hs = np.asarray(hidden_states, f)
    aw = np.asarray(consciousness_vector, f) @ np.asarray(aw_w, f).T \
        + np.asarray(aw_b, f)
    wfused = np.asarray(meta_out_w, f).T @ np.asarray(out_w, f).T  # [D, D]
    xTs = [np.ascontiguousarray(hs[b].T).astype(BF) for b in range(B)]

    def bfT(a):  # transpose + bf16
        return np.ascontiguousarray(np.asarray(a, f).T).astype(BF)

    in_maps = []
    for c in range(NCORES):
        b, g = c // G, c % G
        sl = slice(g * BAND, (g + 1) * BAND)
        wv_aug = np.zeros((D, 260), f)
        for h in range(4):
            wv_aug[:, h * 65:h * 65 + 64] = \
                np.asarray(wv, f)[g * BAND + h * 64: g * BAND + (h + 1) * 64].T
        sel4 = np.zeros((4, 512), f)
        for h in range(4):
            sel4[h, h * 128:(h + 1) * 128] = 1.0
        sel4 = sel4.astype(BF)
        pc = np.zeros((BAND, D), f)
        pc[np.arange(BAND), g * BAND + np.arange(BAND)] = 0.1
        psel = np.zeros((D, BAND), f)
        psel[g * BAND + np.arange(BAND), np.arange(BAND)] = 1.0
        in_maps.append({
            "xT": xTs[b],
            "wqT": bfT(np.asarray(wq, f)[sl] / 8.0),
            "wkT": bfT(np.asarray(wk, f)[sl]),
            "wvT": wv_aug.astype(BF),
            "gwT": bfT(np.asarray(gate_w, f)[4 * g:4 * g + 4]),
            "selT": sel4,
            "awc": np.ascontiguousarray(aw[4 * g:4 * g + 4].reshape(1, 4)),
            "cqT": bfT(np.asarray(causal_in_w, f)[0:D][sl] / 16.0),
            "ckT": bfT(np.asarray(causal_in_w, f)[D:2 * D][sl]),
            "cvT": bfT(np.asarray(causal_in_w, f)[2 * D:][sl]),
            "cowT": np.ascontiguousarray(
                CAUSAL_ACTIVE * np.asarray(causal_out_w, f).T[sl]).astype(BF),
            "pcT": pc.astype(BF),
            "pselT": psel.astype(BF),
            "mqT": bfT(np.asarray(meta_in_w, f)[0:D][sl] / 16.0),
            "mkT": bfT(np.asarray(meta_in_w, f)[D:2 * D][sl]),
            "mvT": bfT(np.asarray(meta_in_w, f)[2 * D:][sl]),
            "mowT": np.ascontiguousarray(MW * wfused[sl]).astype(BF),
            "owT": np.ascontiguousarray(
                (1.0 - MW) * np.asarray(out_w, f).T[sl]).astype(BF),
        })

    nc = _get_nc()
    res = run_bass_kernel_spmd(nc, in_maps, core_ids=list(range(NCORES)))

    bias_row = (np.asarray(out_b, f)
                + MW * (np.asarray(meta_out_b, f) @ np.asarray(out_w, f).T))
    out = np.empty((B, S, D), f)
    for b in range(B):
        acc = np.zeros((D, S), f)
        for g in range(G):
            acc += res.results[b * G + g]["outP"]
        out[b] = acc.T + bias_row[None, :]
    return out


# revision 30
# speedup vs baseline: 1.0060x; 1.0060x over previous
"""Trainium2 8-core kernel for the AGI transformer block.

Sharding: 2-way data parallel over batch x 4-way tensor parallel over heads.
Core c: batch b=c//4, feature band g=c%4 (256 features = 4 main heads of 64 /
1 causal head of 256 / 1 meta head of 256).

Per core (band slice G = [256g, 256g+256)):
  - main attention: 4 heads, q pre-scaled 1/8, sigmoid(gate+aw) modulation
    folded into q per-head; rowsums via ones-column in augmented V (M=65);
    head pairs share the PE array via base-partition 0/64 row packing.
  - causal MHA head: hd=256, q pre-scaled 1/16; 0.9 blend folded into out-proj
    weight; main's 0.1-scaled ctx placed into the out-proj PSUM via a host-built
    placement matrix (keeps the SPMD program core-independent).
  - blend combine: ReduceScatter(add) -> own band [256,2048] (for the final
    0.85-term) + AllGather -> full blended ctx [1024,2048] (for meta).
  - meta MHA head: hd=256; 0.15*meta_out_w.T@out_w.T folded into one matrix.
  - final: outP = mowT.T@metaA + owT.T@band_ctx  (partial; host sums 4 cores).

Storage is bf16 (halves SBUF + HBM traffic); accumulation is f32 in PSUM.
Emission interleaves ACT-bound main attention with PE-bound causal attention
and meta projections so the TensorE stream stays dense (HAM stays warm).
Chunked tiles keep Tile's dependency tracking precise so collectives overlap.
"""

import ml_dtypes
import numpy as np

import concourse.mybir as mybir
import concourse.tile as tile
from concourse import bacc
from concourse.bass_utils import run_bass_kernel_spmd

F32 = mybir.dt.float32
BF16 = mybir.dt.bfloat16
AF = mybir.ActivationFunctionType
BF = ml_dtypes.bfloat16

B, S, D = 2, 2048, 1024
NCORES = 8
G = 4  # tensor-parallel group size
BAND = 256  # features per core
IC, NIC = 512, 4  # i-chunk (query) tiling
NJT = 16  # j tiles of 128
NKT = 8  # contraction tiles of 128 over D
CAUSAL_ACTIVE = 0.9
MW = ((0.9 - 0.8) / 0.2) * 0.3  # 0.15


def build_program():
    nc = bacc.Bacc("TRN2", target_bir_lowering=False, debug=False,
                   num_devices=NCORES)

    def din(name, shape):
        return nc.dram_tensor(name, shape, BF16, kind="ExternalInput").ap()

    xT = din("xT", [D, S])
    wqT = din("wqT", [D, BAND])
    wkT = din("wkT", [D, BAND])
    wvT = din("wvT", [D, 260])  # 4x(64 head cols + zero col for ones)
    gwT = din("gwT", [D, 4])
    selT = din("selT", [4, 512])  # 4 one-hot row-selector blocks [4,128]
    awc = nc.dram_tensor("awc", [1, 4], F32, kind="ExternalInput").ap()
    cqT = din("cqT", [D, BAND])
    ckT = din("ckT", [D, BAND])
    cvT = din("cvT", [D, BAND])
    cowT = din("cowT", [BAND, D])
    pcT = din("pcT", [BAND, D])  # placement matrix (0.1 at own band)
    pselT = din("pselT", [D, BAND])  # one-hot band row-selector (chunk 3)
    mqT = din("mqT", [D, BAND])
    mkT = din("mkT", [D, BAND])
    mvT = din("mvT", [D, BAND])
    mowT = din("mowT", [BAND, D])
    owT = din("owT", [BAND, D])
    outP = nc.dram_tensor("outP", [D, S], F32, kind="ExternalOutput").ap()

    groups = [[0, 1, 2, 3], [4, 5, 6, 7]]

    with tile.TileContext(nc) as tc:
        with (
            tc.tile_pool(name="wts", bufs=1) as wts,
            tc.tile_pool(name="act", bufs=1) as actp,
            tc.tile_pool(name="small", bufs=1) as small,
            tc.tile_pool(name="work", bufs=3) as work,
            tc.tile_pool(name="stat", bufs=2) as statp,
            tc.tile_pool(name="psE", bufs=3, space="PSUM") as psE,
            tc.tile_pool(name="psA", bufs=4, space="PSUM") as psA,
            tc.tile_pool(name="psR", bufs=1, space="PSUM") as psR,
            tc.tile_pool(name="dram", bufs=1, space="DRAM") as dram,
        ):
            def load_w(name, ap, cols, tag):
                t = wts.tile([128, NKT, cols], BF16, name=name, tag=tag)
                for kt in range(NKT):
                    nc.sync.dma_start(t[:, kt, :],
                                      ap[kt * 128:(kt + 1) * 128, :])
                return t

            def load_w2(name, ap, tag):  # [256, 1024] -> [128, 2, 1024]
                t = wts.tile([128, 2, D], BF16, name=name, tag=tag)
                for kt in range(2):
                    nc.sync.dma_start(t[:, kt, :],
                                      ap[kt * 128:(kt + 1) * 128, :])
                return t

            wq_sb = load_w("wq_sb", wqT, BAND, "wq")

            # xT per-kt tiles; tags pair them with later-stage tiles so the
            # SBUF slots time-share (xT dies before those are written)
            xtags = ["ctxC0", "ctxC1", "ctxC2", "ctxC3",
                     "qT2", "kT2", "vA2", "bandC0"]
            xT_t = []
            for kt in range(NKT):
                t = actp.tile([128, S], BF16, name=f"xTt{kt}", tag=xtags[kt])
                nc.sync.dma_start(t[:, :], xT[kt * 128:(kt + 1) * 128, :])
                xT_t.append(t)

            wk_sb = load_w("wk_sb", wkT, BAND, "wk")
            wv_sb = load_w("wv_sb", wvT, 260, "wv")
            gw_sb = load_w("gw_sb", gwT, 4, "gw")
            cq_sb = load_w("cq_sb", cqT, BAND, "cq")
            ck_sb = load_w("ck_sb", ckT, BAND, "ck")
            cv_sb = load_w("cv_sb", cvT, BAND, "cv")
            cow_sb = load_w2("cow_sb", cowT, "cow")
            pc_sb = load_w2("pc_sb", pcT, "pc")
            mow_sb = load_w2("mow_sb", mowT, "mow")  # own slots: load early
            ow_sb = load_w2("ow_sb", owT, "ow")

            aw_sb = small.tile([4, 1], F32)
            nc.sync.dma_start(aw_sb[:, :], awc.rearrange("a b -> b a"))
            sel_sb = small.tile([4, 512], BF16)
            nc.sync.dma_start(sel_sb[:, :], selT[:, :])
            ones_sb = small.tile([128, 1], BF16)
            nc.vector.memset(ones_sb[:, :], 1.0)
            onesrow = small.tile([1, 128], BF16)
            nc.vector.memset(onesrow[:, :], 1.0)

            # ---------- projections ----------
            qT_sb = actp.tile([128, 2, S], BF16, tag="qT")
            kT_sb = actp.tile([128, 2, S], BF16, tag="kT")

            def proj_chunk(dst, w_sb, src_t, ot, icc):
                ps = psA.tile([128, IC], F32, tag="acc")
                for kt in range(NKT):
                    nc.tensor.matmul(
                        ps[:, :],
                        w_sb[:, kt, ot * 128:(ot + 1) * 128],
                        src_t[kt][:, icc * IC:(icc + 1) * IC],
                        start=(kt == 0), stop=(kt == NKT - 1))
                nc.vector.tensor_copy(dst[:, ot, icc * IC:(icc + 1) * IC],
                                      ps[:, :])

            def proj_T(dst, w_sb, src_t):  # dst [128, 2, S]
                for ot in range(2):
                    for icc in range(NIC):
                        proj_chunk(dst, w_sb, src_t, ot, icc)

            # gate matmuls + sigmoid for ALL chunks now (keeps the sigmoid
            # table-set switch out of the attention blocks)
            mrow4 = small.tile([4, S], BF16)
            for icc in range(NIC):
                i0 = icc * IC
                g_ps = psR.tile([4, IC], F32, tag="rs")
                for kt in range(NKT):
                    nc.tensor.matmul(g_ps[:, :],
                                     gw_sb[:, kt, 0:4],
                                     xT_t[kt][:, i0:i0 + IC],
                                     start=(kt == 0), stop=(kt == NKT - 1))
                nc.scalar.activation(mrow4[:, i0:i0 + IC], g_ps[:, :],
                                     AF.Sigmoid, bias=aw_sb[:, 0:1], scale=1.0)

            qs_sb = actp.tile([128, 2, 2 * IC], BF16, tag="qs")  # 2-chunk ring

            def qmod(h, icc):
                # broadcast row h of mrow4 to 128 partitions via a K=4 matmul
                # against a host-provided one-hot selector, then scale q into
                # a separate tile (avoids write-after-read on qT)
                rh, oh = (h % 2) * 64, h // 2
                i0 = icc * IC
                pb = psR.tile([128, IC], F32, tag="rs")
                nc.tensor.matmul(pb[:, :],
                                 sel_sb[0:4, h * 128:(h + 1) * 128],
                                 mrow4[0:4, i0:i0 + IC])
                r0 = (icc % 2) * IC
                nc.vector.tensor_mul(qs_sb[rh:rh + 64, oh, r0:r0 + IC],
                                     qT_sb[rh:rh + 64, oh, i0:i0 + IC],
                                     pb[rh:rh + 64, :])

            def qproj_steps(icc):
                """q + cq projection/modulation filler steps for chunk icc"""
                steps = []
                for ot in range(2):
                    def sq(ot=ot, icc=icc):
                        proj_chunk(qT_sb, wq_sb, xT_t, ot, icc)
                        qmod(2 * ot, icc)
                        qmod(2 * ot + 1, icc)
                    steps.append(sq)
                for ot in range(2):
                    steps.append(lambda ot=ot, icc=icc: proj_chunk(
                        cqT_sb, cq_sb, xT_t, ot, icc))
                return steps

            # stage B: only chunk 0 of q/cq; full k/v/ck/cv
            cqT_sb = actp.tile([128, 2, S], BF16, tag="cqT")
            for st_ in qproj_steps(0):
                st_()
            proj_T(kT_sb, wk_sb, xT_t)

            # v natural layout [2048 j, 260]: ones cols at 64,129,194,259
            v_sb = actp.tile([128, NJT, 260], BF16, tag="vA")
            for st in range(NJT):
                ps = psA.tile([128, 260], F32, tag="acc")
                for kt in range(NKT):
                    nc.tensor.matmul(ps[:, :],
                                     xT_t[kt][:, st * 128:(st + 1) * 128],
                                     wv_sb[:, kt, :],
                                     start=(kt == 0), stop=(kt == NKT - 1))
                nc.vector.tensor_copy(v_sb[:, st, :], ps[:, :])
                nc.vector.memset(v_sb[:, st, 64:260:65], 1.0)

            ckT_sb = actp.tile([128, 2, S], BF16, tag="ckT")
            proj_T(ckT_sb, ck_sb, xT_t)

            cv_nat = actp.tile([128, NJT, BAND], BF16, tag="cvN")
            for st in range(NJT):
                ps = psA.tile([128, BAND], F32, tag="acc")
                for kt in range(NKT):
                    nc.tensor.matmul(ps[:, :],
                                     xT_t[kt][:, st * 128:(st + 1) * 128],
                                     cv_sb[:, kt, :],
                                     start=(kt == 0), stop=(kt == NKT - 1))
                nc.vector.tensor_copy(cv_nat[:, st, :], ps[:, :])

            # meta weights: load now (slots of wq/wk/wv just freed; sync queue
            # still shallow, so they land long before the meta stage)
            mq_sb = load_w("mq_sb", mqT, BAND, "wq")
            mk_sb = load_w("mk_sb", mkT, BAND, "wk")
            mv_sb = load_w("mv_sb", mvT, BAND, "wv")

            # ---------- chunked tiles ----------
            ctxm_sb = actp.tile([128, 2, S], BF16, tag="ctxm")  # 0.1*main ctx
            cA_sb = actp.tile([128, 2, S], BF16, tag="cA")
            ctxC = [actp.tile([128, NKT, IC], BF16, name=f"ctxC{i}",
                              tag=f"ctxC{i}") for i in range(NIC)]
            bandC = [actp.tile([128, 2, IC], BF16, name=f"bandC{i}",
                               tag=f"bandC{i}") for i in range(NIC)]
            mqT_sb = actp.tile([128, 2, S], BF16, tag="qT2")
            mkT_sb = actp.tile([128, 2, S], BF16, tag="kT2")
            mv_nat = actp.tile([128, NJT, BAND], BF16, tag="vA2")

            arB, rsO, agO, arO3 = [], [], [], []
            for icc in range(NIC):
                nh = 1 if icc < NIC - 1 else 2
                arB.append([dram.tile([D, IC // nh], BF16,
                                      name=f"arB{icc}_{hh}", tag=f"arB{icc}{hh}")
                            for hh in range(nh)])
                rsO.append([dram.tile([BAND, IC // nh], BF16,
                                      name=f"rsO{icc}_{hh}", tag=f"rsO{icc}{hh}")
                            for hh in range(nh)])
                if icc == NIC - 1:
                    arO3.extend([dram.tile([D, IC // nh], BF16,
                                           name=f"arO3_{hh}", tag=f"arO3{hh}")
                                 for hh in range(nh)])
                agO.append([dram.tile([D, IC // nh], BF16,
                                      name=f"agO{icc}_{hh}", tag=f"agO{icc}{hh}")
                            for hh in range(nh)])

            def div_batch(specs, i0):
                """batched softmax divisions: pipeline ln -> exp -> bcast ->
                muls across several heads so chain latencies overlap.
                spec: ("head", h, acc) or ("wide", dst_sb, a1, a2, rs)"""
                lnrs = []
                for sp in specs:
                    lnr = statp.tile([1, IC], F32, tag="lnr", bufs=3)
                    src = sp[2][64:65, :] if sp[0] == "head" else sp[4][:, :]
                    nc.scalar.activation(lnr[:, :], src, AF.Ln)
                    lnrs.append(lnr)
                rcps = []
                for sp, lnr in zip(specs, lnrs):
                    rcp = statp.tile([1, IC], BF16, tag="rcp", bufs=3)
                    nc.scalar.activation(rcp[:, :], lnr[:, :], AF.Exp,
                                         scale=-1.0)
                    rcps.append(rcp)
                pbs = []
                for sp, rcp in zip(specs, rcps):
                    n = 64 if sp[0] == "head" else 128
                    # wide-head broadcasts use the psR bank (freed by the ln
                    # that read the rowsum); head broadcasts rotate via eps
                    if sp[0] == "head":
                        pb_ps = psE.tile([128, IC], F32, tag="eps")
                    else:
                        pb_ps = psR.tile([128, IC], F32, tag="rs")
                    nc.tensor.matmul(pb_ps[:, :], onesrow[0:1, :], rcp[:, :])
                    pb = work.tile([n, IC], BF16,
                                   tag="pbm" if n == 64 else "pb2", bufs=3)
                    nc.scalar.copy(pb[:, :], pb_ps[0:n, :])
                    pbs.append(pb)
                for sp, pb in zip(specs, pbs):
                    if sp[0] == "head":
                        h, acc = sp[1], sp[2]
                        rh, oh = (h % 2) * 64, h // 2
                        nc.vector.tensor_mul(
                            ctxm_sb[rh:rh + 64, oh, i0:i0 + IC],
                            acc[0:64, :], pb[:, :])
                    else:
                        dst_sb, a1, a2 = sp[1], sp[2], sp[3]
                        nc.vector.tensor_mul(dst_sb[:, 0, i0:i0 + IC],
                                             a1[:, :], pb[:, :])
                        nc.vector.tensor_mul(dst_sb[:, 1, i0:i0 + IC],
                                             a2[:, :], pb[:, :])

            def main_pair_step(p, jt, i0, accs):
                """one j-tile for main head pair p (heads 2p, 2p+1): the two
                E matmuls row-pack (base partitions 0/64) and run concurrently"""
                oh = p
                esbs = []
                for hh in range(2):
                    rh = hh * 64
                    eps = psE.tile([128, IC], F32, tag="eps")
                    r0 = (i0 // IC % 2) * IC
                    nc.tensor.matmul(
                        eps[:, :],
                        kT_sb[rh:rh + 64, oh, jt * 128:(jt + 1) * 128],
                        qs_sb[rh:rh + 64, oh, r0:r0 + IC])
                    esb = work.tile([128, IC], BF16, tag="esb", bufs=4)
                    nc.scalar.activation(esb[:, :], eps[:, :], AF.Exp)
                    esbs.append(esb)
                for hh in range(2):
                    h = 2 * p + hh
                    nc.tensor.matmul(
                        accs[hh][:, :],
                        v_sb[:, jt, h * 65:h * 65 + 65],
                        esbs[hh][:, :],
                        start=(jt == 0), stop=(jt == NJT - 1))

            def wide_attn_step(kTt, qTt, vnat, jt, i0, a1, a2, rs):
                """one j-tile of a hd-256 attention (causal or meta)"""
                eps = psE.tile([128, IC], F32, tag="eps")
                for dkt in range(2):
                    nc.tensor.matmul(
                        eps[:, :],
                        kTt[:, dkt, jt * 128:(jt + 1) * 128],
                        qTt[:, dkt, i0:i0 + IC],
                        start=(dkt == 0), stop=(dkt == 1))
                esb = work.tile([128, IC], BF16, tag="esb", bufs=4)
                nc.scalar.activation(esb[:, :], eps[:, :], AF.Exp)
                st_, sp_ = (jt == 0), (jt == NJT - 1)
                nc.tensor.matmul(a1[:, :], vnat[:, jt, 0:128], esb[:, :],
                                 start=st_, stop=sp_)
                nc.tensor.matmul(a2[:, :], vnat[:, jt, 128:256], esb[:, :],
                                 start=st_, stop=sp_)
                nc.tensor.matmul(rs[:, :], ones_sb[:, 0:1], esb[:, :],
                                 start=st_, stop=sp_)

            def metaproj_steps(icc):
                """closures, each emitting one PSUM group of chunk icc's meta
                projections (interleaved into phase 2 of chunk icc+1)"""
                i0 = icc * IC
                steps = []

                def projstep(w_sb, dst, ot, i0=i0, icc=icc):
                    ps = psA.tile([128, IC], F32, tag="acc")
                    for kt in range(NKT):
                        nc.tensor.matmul(
                            ps[:, :], w_sb[:, kt, ot * 128:(ot + 1) * 128],
                            ctxC[icc][:, kt, :],
                            start=(kt == 0), stop=(kt == NKT - 1))
                    nc.vector.tensor_copy(dst[:, ot, i0:i0 + IC], ps[:, :])

                def vstep(st4, icc=icc):
                    st = icc * 4 + st4
                    ps = psA.tile([128, BAND], F32, tag="acc")
                    for kt in range(NKT):
                        nc.tensor.matmul(
                            ps[:, :],
                            ctxC[icc][:, kt, st4 * 128:(st4 + 1) * 128],
                            mv_sb[:, kt, :],
                            start=(kt == 0), stop=(kt == NKT - 1))
                    nc.vector.tensor_copy(mv_nat[:, st, :], ps[:, :])

                for ot in range(2):
                    steps.append(lambda ot=ot: projstep(mq_sb, mqT_sb, ot))
                for ot in range(2):
                    steps.append(lambda ot=ot: projstep(mk_sb, mkT_sb, ot))
                for st4 in range(4):
                    steps.append(lambda st4=st4: vstep(st4))
                return steps

            # ---------- per-chunk pipeline ----------
            pending_rdback = []
            for icc in range(NIC):
                i0 = icc * IC
                while pending_rdback:
                    pending_rdback.pop(0)()
                # phase 1: main heads (0,1) interleaved with causal attention
                accA = [psA.tile([65, IC], F32, tag="acc", name=f"accA{icc}{i}")
                        for i in range(2)]
                ca1 = psA.tile([128, IC], F32, tag="acc")
                ca2 = psA.tile([128, IC], F32, tag="acc")
                crs = psR.tile([1, IC], F32, tag="rs")
                for jt in range(NJT):
                    main_pair_step(0, jt, i0, accA)
                    wide_attn_step(ckT_sb, cqT_sb, cv_nat, jt, i0, ca1, ca2, crs)
                div_batch([("head", 0, accA[0]), ("head", 1, accA[1]),
                           ("wide", cA_sb, ca1, ca2, crs)], i0)

                # phase 2: main heads (2,3) interleaved with filler PE work:
                # next chunk's q/cq projections + older chunk's meta projs
                accB = [psA.tile([65, IC], F32, tag="acc", name=f"accB{icc}{i}")
                        for i in range(2)]
                fillers = []
                if icc + 1 < NIC:
                    fillers += qproj_steps(icc + 1)
                if icc >= 2:
                    fillers += metaproj_steps(icc - 2)
                # weave ~2/3 of the fillers into rounds 0..11 so the pair-B
                # accumulators stop promptly; the rest interleave with the
                # causal out-proj below (keeps PE dense across the boundary)
                nfront = len(fillers)
                done = 0
                for jt in range(NJT):
                    main_pair_step(1, jt, i0, accB)
                    want = (jt + 1) * nfront // NJT
                    while done < want:
                        fillers[done]()
                        done += 1
                div_batch([("head", 2, accB[0]), ("head", 3, accB[1])], i0)

                # causal out-proj + main placement -> arB chunk
                for ot in range(8):
                    ps = psA.tile([128, IC], F32, tag="acc")
                    for ft in range(2):
                        nc.tensor.matmul(
                            ps[:, :],
                            cow_sb[:, ft, ot * 128:(ot + 1) * 128],
                            cA_sb[:, ft, i0:i0 + IC],
                            start=(ft == 0), stop=False)
                    for rt in range(2):
                        nc.tensor.matmul(
                            ps[:, :],
                            pc_sb[:, rt, ot * 128:(ot + 1) * 128],
                            ctxm_sb[:, rt, i0:i0 + IC],
                            start=False, stop=(rt == 1))
                    ob = work.tile([128, IC], BF16, tag="obA", bufs=2)
                    nc.vector.tensor_copy(ob[:, :], ps[:, :])
                    nh = len(arB[icc])
                    hw_ = IC // nh
                    for hh in range(nh):
                        nc.sync.dma_start(
                            arB[icc][hh][ot * 128:(ot + 1) * 128, :],
                            ob[:, hh * hw_:(hh + 1) * hw_])
                    if done < len(fillers) and ot % 2 == 1:
                        fillers[done]()
                        done += 1

                # blend combine: RS (own band) + AG (full ctx); the last
                # chunk is split in half so its latency tail is shorter.
                # Read-back DMAs are DEFERRED one block so the sync queue
                # never parks on an unfinished collective (head-of-line).
                nh = len(arB[icc])
                hw_ = IC // nh
                for hh in range(nh):
                    c0 = hh * hw_
                    if icc < NIC - 1:
                        nc.gpsimd.collective_compute(
                            "ReduceScatter", mybir.AluOpType.add,
                            replica_groups=groups,
                            ins=[arB[icc][hh][:, :].opt()],
                            outs=[rsO[icc][hh][:, :].opt()])
                        nc.gpsimd.collective_compute(
                            "AllGather", mybir.AluOpType.bypass,
                            replica_groups=groups,
                            ins=[rsO[icc][hh][:, :].opt()],
                            outs=[agO[icc][hh][:, :].opt()])
                    else:
                        # last chunk: single AllReduce per half (shorter
                        # serial chain on the collective engine); own band
                        # recovered by a select matmul in the tail
                        nc.gpsimd.collective_compute(
                            "AllReduce", mybir.AluOpType.add,
                            replica_groups=groups,
                            ins=[arB[icc][hh][:, :].opt()],
                            outs=[arO3[hh][:, :].opt()])

                    def rdback(icc=icc, hh=hh, c0=c0, hw_=hw_):
                        src = agO[icc][hh] if icc < NIC - 1 else arO3[hh]
                        for kt in range(NKT):
                            nc.sync.dma_start(
                                ctxC[icc][:, kt, c0:c0 + hw_],
                                src[kt * 128:(kt + 1) * 128, :])
                        if icc < NIC - 1:
                            for kt in range(2):
                                nc.sync.dma_start(
                                    bandC[icc][:, kt, c0:c0 + hw_],
                                    rsO[icc][hh][kt * 128:(kt + 1) * 128, :])
                    pending_rdback.append(rdback)

            # meta projections for the last two chunks (chunk 2 overlaps
            # the RS3/AG3 tail; chunk 3 is emitted inside the first meta
            # attention chunk below, after its early j-tiles)
            while pending_rdback:
                pending_rdback.pop(0)()
            for st in metaproj_steps(NIC - 2):
                st()

            # ---------- meta attention + final out-proj ----------
            def final_steps(icc):
                i0 = icc * IC
                steps = []

                def fstep(ot, icc=icc, i0=i0):
                    ps = psA.tile([128, IC], F32, tag="acc")
                    for ft in range(2):
                        nc.tensor.matmul(
                            ps[:, :],
                            mow_sb[:, ft, ot * 128:(ot + 1) * 128],
                            mA_sb[:, ft, i0:i0 + IC],
                            start=(ft == 0), stop=False)
                    for ft in range(2):
                        nc.tensor.matmul(
                            ps[:, :],
                            ow_sb[:, ft, ot * 128:(ot + 1) * 128],
                            bandC[icc][:, ft, :],
                            start=False, stop=(ft == 1))
                    ob = work.tile([128, IC], F32, tag="obF", bufs=2)
                    nc.vector.tensor_copy(ob[:, :], ps[:, :])
                    nc.sync.dma_start(
                        outP[ot * 128:(ot + 1) * 128, i0:i0 + IC], ob[:, :])
                for ot in range(8):
                    steps.append(lambda ot=ot: fstep(ot))
                return steps

            mA_sb = actp.tile([128, 2, S], BF16, tag="cqT")  # reuse slot
            for icc in range(NIC):
                i0 = icc * IC
                fsteps = final_steps(icc - 1) if icc > 0 else []
                a1 = psA.tile([128, IC], F32, tag="acc")
                a2 = psA.tile([128, IC], F32, tag="acc")
                rs = psR.tile([1, IC], F32, tag="rs")
                # j-tiles 0..11 only touch chunks 0-2 of mk/mv, so the first
                # i-chunk's early j-tiles run while chunk 3's AG completes;
                # chunk 3's meta projections emit before the last 4 j-tiles
                for jt in range(12):
                    wide_attn_step(mkT_sb, mqT_sb, mv_nat, jt, i0, a1, a2, rs)
                    if fsteps and jt % 2 == 1 and jt // 2 < len(fsteps):
                        fsteps[jt // 2]()
                if icc == 0:
                    psel_sb = load_w("psel_sb", pselT, BAND, "cq")
                    for st in metaproj_steps(NIC - 1):
                        st()
                    for rt in range(2):
                        ps = psA.tile([128, IC], F32, tag="acc")
                        for kt in range(NKT):
                            nc.tensor.matmul(
                                ps[:, :],
                                psel_sb[:, kt, rt * 128:(rt + 1) * 128],
                                ctxC[NIC - 1][:, kt, :],
                                start=(kt == 0), stop=(kt == NKT - 1))
                        nc.vector.tensor_copy(bandC[NIC - 1][:, rt, :],
                                              ps[:, :])
                for jt in range(12, NJT):
                    wide_attn_step(mkT_sb, mqT_sb, mv_nat, jt, i0, a1, a2, rs)
                    if fsteps and jt % 2 == 1 and jt // 2 < len(fsteps):
                        fsteps[jt // 2]()
                div_batch([("wide", mA_sb, a1, a2, rs)], i0)

            for st in final_steps(NIC - 1):
                st()

    nc.compile()
    return nc


_NC = None


def _get_nc():
    global _NC
    if _NC is None:
        _NC = build_program()
    return _NC


def kernel(hidden_states, consciousness_vector, wq, bq, wk, bk, wv, bv,
           gate_w, gate_b, aw_w, aw_b,
           causal_in_w, causal_in_b, causal_out_w, causal_out_b,
           meta_in_w, meta_in_b, meta_out_w, meta_out_b,
           out_w, out_b):
    f = np.float32
    hs = np.asarray(hidden_states, f)
    aw = np.asarray(consciousness_vector, f) @ np.asarray(aw_w, f).T \
        + np.asarray(aw_b, f)
    wfused = np.asarray(meta_out_w, f).T @ np.asarray(out_w, f).T  # [D, D]
    xTs = [np.ascontiguousarray(hs[b].T).astype(BF) for b in range(B)]

    def bfT(a):  # transpose + bf16
        return np.ascontiguousarray(np.asarray(a, f).T).astype(BF)

    in_maps = []
    for c in range(NCORES):
        b, g = c // G, c % G
        sl = slice(g * BAND, (g + 1) * BAND)
        wv_aug = np.zeros((D, 260), f)
        for h in range(4):
            wv_aug[:, h * 65:h * 65 + 64] = \
                np.asarray(wv, f)[g * BAND + h * 64: g * BAND + (h + 1) * 64].T
        sel4 = np.zeros((4, 512), f)
        for h in range(4):
            sel4[h, h * 128:(h + 1) * 128] = 1.0
        sel4 = sel4.astype(BF)
        pc = np.zeros((BAND, D), f)
        pc[np.arange(BAND), g * BAND + np.arange(BAND)] = 0.1
        psel = np.zeros((D, BAND), f)
        psel[g * BAND + np.arange(BAND), np.arange(BAND)] = 1.0
        in_maps.append({
            "xT": xTs[b],
            "wqT": bfT(np.asarray(wq, f)[sl] / 8.0),
            "wkT": bfT(np.asarray(wk, f)[sl]),
            "wvT": wv_aug.astype(BF),
            "gwT": bfT(np.asarray(gate_w, f)[4 * g:4 * g + 4]),
            "selT": sel4,
            "awc": np.ascontiguousarray(aw[4 * g:4 * g + 4].reshape(1, 4)),
            "cqT": bfT(np.asarray(causal_in_w, f)[0:D][sl] / 16.0),
            "ckT": bfT(np.asarray(causal_in_w, f)[D:2 * D][sl]),
            "cvT": bfT(np.asarray(causal_in_w, f)[2 * D:][sl]),
            "cowT": np.ascontiguousarray(
                CAUSAL_ACTIVE * np.asarray(causal_out_w, f).T[sl]).astype(BF),
            "pcT": pc.astype(BF),
            "pselT": psel.astype(BF),
            "mqT": bfT(np.asarray(meta_in_w, f)[0:D][sl] / 16.0),
            "mkT": bfT(np.asarray(meta_in_w, f)[D:2 * D][sl]),
            "mvT": bfT(np.asarray(meta_in_w, f)[2 * D:][sl]),
            "mowT": np.ascontiguousarray(MW * wfused[sl]).astype(BF),
            "owT": np.ascontiguousarray(
                (1.0 - MW) * np.asarray(out_w, f).T[sl]).astype(BF),
        })

    nc = _get_nc()
    res = run_bass_kernel_spmd(nc, in_maps, core_ids=list(range(NCORES)))

    bias_row = (np.asarray(out_b, f)
                + MW * (np.asarray(meta_out_b, f) @ np.asarray(out_w, f).T))
    out = np.empty((B, S, D), f)
    for b in range(B):
        acc = np.zeros((D, S), f)
        for g in range(G):
            acc += res.results[b * G + g]["outP"]
        out[b] = acc.T + bias_row[None, :]
    return out
